# revision 26
# baseline (speedup 1.0000x reference)
"""Trainium2 Bass kernel for nn_ModMultiHeadAttentionLayer (sparse_attention).

Math notes
----------
reference per batch b:
  tc_p     = LayerNorm(code @ wc_w_p.T + wc_b_p) * ln_g + ln_b          (p in q,k,v,o)
  P(x)     = (tc_p * x) @ W.T + b  ==  x @ (W * tc_p).T + b             (fold tc into W)
  energy   = Q K^T / 8 per head (HD=64), A = softmax_j(energy)
  att      = (u_i u_j A) / (1e-6 + u_i u_j A)
           = sigmoid( energy/8 + ln u_i + ln u_j - ln S_i - ln 1e-6 )   (S_i = sum_j exp(energy/8))
  x        = att @ V ; out = P_o(x)

Device strategy (1 batch element per NeuronCore, 8 cores):
  * host folds tc into the weights, transposes activations, casts to bf16
  * projections produce Q^T, K^T (transposed) and V (natural) via PE
  * phase 2: energy[i,j] per head -> ACT exp (scale=1/8) with accum_out -> S_i
  * gamma row: 8*(ln u_i - ln S_i - mhat) computed on-chip, bounced through
    DRAM to convert partition-major -> row layout, cast fp16
  * phase 3: energy^T[j,i] + rank-1 fp16 matmul injects the per-i term
    (gamma, from a DVE bit-trick log2 of S); ACT tanh with per-partition
    bias emits th; DVE affine 0.5*th+0.5 emits att^T fp32 (DMA out) and
    bf16 (PV matmul, V as lhsT). sigma(z) = 0.5*tanh(z/2)+0.5 keeps Exp
    and Tanh in one ACT table set (no table reloads).
  * all phases software-pipelined per head (phase-3 lags by 3 heads so
    its K=64 matmuls sit on the opposite PE row-group parity from the
    phase-2 matmuls next to them -> concurrent on HW via row tiling)
  * phase 4: output projection from x_att^T
  * host returns attention as a transposed view (device wrote [b,h,j,i])
"""

import sys

for _p in ("/opt/trn_rl_repo", "/root/.axon_site/_ro/trn_rl_repo"):
    if _p not in sys.path:
        sys.path.append(_p)

import numpy as np
import ml_dtypes

import concourse.mybir as mybir
import concourse.bacc as bacc
import concourse.tile as tile
from concourse.bass_utils import run_bass_kernel_spmd

B, S, D, H, CD = 8, 1024, 1024, 16, 512
HD = D // H            # 64
P = 128
NCHUNK = D // P        # 8
LN_EPS = 1e-5
LOG1E6 = 13.815510557964274   # -ln(1e-6)

f32 = mybir.dt.float32
bf16 = mybir.dt.bfloat16
f16 = mybir.dt.float16
AF = mybir.ActivationFunctionType
OP = mybir.AluOpType

LAST_RESULT = None
_CACHED_NC = None


def _build_bass(ln_batch=1):
    """Head-pipelined build: projections, exp/S pass, and the tanh/PV pass
    are interleaved per head. All ACT transcendentals (Exp, Tanh) live in
    one table set -> a single table load. ln(S) runs on DVE via a bit-trick
    quadratic log2 (max ln err ~7e-4)."""
    del ln_batch
    nc = bacc.Bacc(None, target_bir_lowering=False)

    qT = nc.dram_tensor("qT", [D, S], bf16, kind="ExternalInput")
    kT = nc.dram_tensor("kT", [D, S], bf16, kind="ExternalInput")
    vT = nc.dram_tensor("vT", [D, S], bf16, kind="ExternalInput")
    wq = nc.dram_tensor("wq", [D, D], bf16, kind="ExternalInput")
    wk = nc.dram_tensor("wk", [D, D], bf16, kind="ExternalInput")
    wv = nc.dram_tensor("wv", [D, D], bf16, kind="ExternalInput")
    wo = nc.dram_tensor("wo", [D, D], bf16, kind="ExternalInput")
    biasj_d = nc.dram_tensor("biasj", [P, NCHUNK], f32, kind="ExternalInput")
    lnu8c_d = nc.dram_tensor("lnu8c", [P, NCHUNK], f32, kind="ExternalInput")

    attT_d = nc.dram_tensor("attT", [H, S, S], f32, kind="ExternalOutput")
    xout_d = nc.dram_tensor("xout", [S, D], f32, kind="ExternalOutput")

    i32 = mybir.dt.int32
    FL_A, FL_B = 0.4208645374301046, -0.1563861133763677
    LN2_8 = 8.0 * 0.6931471805599453

    with tile.TileContext(nc) as tc:
        with tc.tile_pool(name="const", bufs=1) as constp, \
             tc.tile_pool(name="xT", bufs=16) as xTp, \
             tc.tile_pool(name="w", bufs=3) as wp, \
             tc.tile_pool(name="big", bufs=1) as bigp, \
             tc.tile_pool(name="small", bufs=3) as smallp, \
             tc.tile_pool(name="escr", bufs=1) as escrp, \
             tc.tile_pool(name="th", bufs=2) as thp, \
             tc.tile_pool(name="att", bufs=2) as attp, \
             tc.tile_pool(name="attb", bufs=2) as attbp, \
             tc.tile_pool(name="xo", bufs=2) as xop, \
             tc.tile_pool(name="vstage", bufs=8) as vstp, \
             tc.tile_pool(name="gdram", bufs=4, space="DRAM") as gdram, \
             tc.tile_pool(name="psA", bufs=3, space="PSUM") as psA, \
             tc.tile_pool(name="pspv", bufs=1, space="PSUM") as pspvp:

            biasj = constp.tile([P, NCHUNK], f32)
            nc.sync.dma_start(biasj[:], biasj_d[:])
            lnu8c = constp.tile([P, NCHUNK], f32)
            nc.sync.dma_start(lnu8c[:], lnu8c_d[:])
            ones = constp.tile([P, P], f16)
            nc.vector.memset(ones[:], 1.0)

            QT = bigp.tile([P, NCHUNK, S], bf16, tag="QT")
            KT = bigp.tile([P, NCHUNK, S], bf16, tag="KT")
            V = bigp.tile([P, NCHUNK, D], bf16, tag="V")
            XA = bigp.tile([P, NCHUNK, S], bf16, tag="XA")
            S_all = bigp.tile([P, H, NCHUNK], f32, tag="S_all")
            grow = bigp.tile([P, H // 2, S], f16, tag="grow")  # head h at partition (h%2)*64, col h//2

            wq_t = wp.tile([P, NCHUNK, D], bf16, tag="w")
            for kc in range(NCHUNK):
                nc.sync.dma_start(wq_t[:, kc, :], wq[kc * P:(kc + 1) * P, :])
            qc, kc_t = [], []
            for kc in range(NCHUNK):
                t = xTp.tile([P, S], bf16, tag="xT")
                nc.sync.dma_start(t[:], qT[kc * P:(kc + 1) * P, :])
                qc.append(t)
            wk_t = wp.tile([P, NCHUNK, D], bf16, tag="w")
            for kc in range(NCHUNK):
                nc.sync.dma_start(wk_t[:, kc, :], wk[kc * P:(kc + 1) * P, :])
            for kc in range(NCHUNK):
                t = xTp.tile([P, S], bf16, tag="xT")
                nc.sync.dma_start(t[:], kT[kc * P:(kc + 1) * P, :])
                kc_t.append(t)

            def _proj_chunk(w_t, xc, dst, dc):
                ps = psA.tile([P, S], f32, tag="ps")
                for kc in range(NCHUNK):
                    for st in range(2):
                        nc.tensor.matmul(
                            ps[:, st * 512:(st + 1) * 512],
                            w_t[:, kc, dc * P:(dc + 1) * P],
                            xc[kc][:, st * 512:(st + 1) * 512],
                            start=(kc == 0), stop=(kc == NCHUNK - 1))
                nc.vector.tensor_copy(dst[:, dc, :], ps[:])

            def _ph2_unit(h, ic):
                hb = (h % 2) * HD
                dc = h // 2
                ps = psA.tile([P, S], f32, tag="ps", name=f"ps_p2_{h}_{ic}")
                for jt in range(2):
                    nc.tensor.matmul(
                        ps[:, jt * 512:(jt + 1) * 512],
                        QT[hb:hb + HD, dc, ic * P:(ic + 1) * P],
                        KT[hb:hb + HD, dc, jt * 512:(jt + 1) * 512],
                        start=True, stop=True)
                e = escrp.tile([P, S], bf16, tag="escr", name=f"e_{h}_{ic}")
                nc.scalar.activation(e[:], ps[:], AF.Exp, scale=0.125,
                                     accum_out=S_all[:, h, ic:ic + 1])

            def _gamma(h):
                # gam = -8*lnS + lnu8c = -8*ln2*log2(S) + 8*(ln u_i - mhat)
                sv = S_all[:, h, :]
                iv = sv.bitcast(i32)
                t = smallp.tile([P, NCHUNK], f32, tag="t")
                nc.vector.tensor_scalar(t[:], iv, 2.0 ** -23, -127.0,
                                        OP.mult, OP.add)
                mi = smallp.tile([P, NCHUNK], i32, tag="mi")
                nc.vector.tensor_scalar(mi[:], iv, 0x007FFFFF, None,
                                        OP.bitwise_and)
                m = smallp.tile([P, NCHUNK], f32, tag="m")
                nc.vector.tensor_scalar(m[:], mi[:], 2.0 ** -23, None, OP.mult)
                hh = smallp.tile([P, NCHUNK], f32, tag="hh")
                nc.vector.tensor_scalar(hh[:], m[:], FL_B, FL_A, OP.mult, OP.add)
                q = smallp.tile([P, NCHUNK], f32, tag="q")
                nc.vector.tensor_scalar(q[:], m[:], -1.0, 1.0, OP.mult, OP.add)
                nc.vector.tensor_tensor(q[:], q[:], m[:], OP.mult)
                nc.vector.tensor_tensor(q[:], q[:], hh[:], OP.mult)
                nc.vector.tensor_tensor(t[:], t[:], q[:], OP.add)  # log2 S
                gam = smallp.tile([P, NCHUNK], f16, tag="gam")
                nc.vector.scalar_tensor_tensor(gam[:], t[:], -LN2_8, lnu8c[:],
                                               OP.mult, OP.add)
                scr = gdram.tile([NCHUNK, P], f16)
                nc.sync.dma_start(scr[:].rearrange("c p -> p c"), gam[:])
                gp = (h % 2) * 64
                nc.sync.dma_start(grow[gp:gp + 1, h // 2, :],
                                  scr[:].rearrange("c p -> (c p)")[None, :])

            pvps_box = [None]
            pv_queue = []   # (h, n, attb) awaiting the PV matmul

            def _pv_flush(limit):
                while len(pv_queue) > limit:
                    h, jc, attb = pv_queue.pop(0)
                    hb = (h % 2) * HD
                    for it in range(2):
                        sl = slice(it * 512, (it + 1) * 512)
                        nc.tensor.matmul(
                            pvps_box[0][hb:hb + HD, sl],
                            V[:, jc, h * HD:(h + 1) * HD],
                            attb[:, sl],
                            start=(jc == 0), stop=(jc == NCHUNK - 1))

            def _ph3_unit(h, jc):
                # emits energy^T+gamma matmuls for both 512-halves, one tanh,
                # affine casts; queues the PV matmuls (drained lagged)
                hb = (h % 2) * HD
                dc = h // 2
                if jc == 0 and h % 2 == 0:
                    pvps_box[0] = pspvp.tile([P, S], f32, tag="pvps",
                                             name=f"pvps_{h}")
                ps = psA.tile([P, S], f32, tag="ps", name=f"ps_p3_{h}_{jc}")
                gp = (h % 2) * 64
                for it in range(2):
                    sl = slice(it * 512, (it + 1) * 512)
                    nc.tensor.matmul(
                        ps[:, sl],
                        KT[hb:hb + HD, dc, jc * P:(jc + 1) * P],
                        QT[hb:hb + HD, dc, sl],
                        start=True, stop=False)
                    nc.tensor.matmul(
                        ps[:, sl], ones[gp:gp + 1, :], grow[gp:gp + 1, h // 2, sl],
                        start=False, stop=True)
                _pv_flush(2)
                th = thp.tile([P, S], f32, tag="th", name=f"th_{h}_{jc}")
                nc.scalar.activation(th[:], ps[:], AF.Tanh,
                                     bias=biasj[:, jc:jc + 1], scale=0.0625)
                att = attp.tile([P, S], f32, tag="att", name=f"att_{h}_{jc}")
                nc.vector.tensor_scalar(att[:], th[:], 0.5, 0.5, OP.mult, OP.add)
                nc.sync.dma_start(attT_d[h, jc * P:(jc + 1) * P, :], att[:])
                attb = attbp.tile([P, S], bf16, tag="attb", name=f"attb_{h}_{jc}")
                nc.vector.tensor_scalar(attb[:], th[:], 0.5, 0.5, OP.mult, OP.add)
                pv_queue.append((h, jc, attb))

            def _ph3_tail(h):
                _pv_flush(0)
                if h % 2 == 1:
                    nc.vector.tensor_copy(XA[:, h // 2, :], pvps_box[0][:])

            wv_t = wo_t = None
            vc = []
            for h in range(H + 3):
                dc = h // 2
                if h < H and h % 2 == 0:
                    _proj_chunk(wq_t, qc, QT, dc)
                    _proj_chunk(wk_t, kc_t, KT, dc)
                for k in range(NCHUNK):
                    if h in (2, 3) and k % 2 == 0:
                        sc = (h - 2) * 4 + k // 2
                        ps = psA.tile([P, S], f32, tag="ps", name=f"ps_v_{sc}")
                        for kc in range(NCHUNK):
                            for dt in range(2):
                                nc.tensor.matmul(
                                    ps[:, dt * 512:(dt + 1) * 512],
                                    vc[kc][:, sc * P:(sc + 1) * P],
                                    wv_t[:, kc, dt * 512:(dt + 1) * 512],
                                    start=(kc == 0), stop=(kc == NCHUNK - 1))
                        nc.vector.tensor_copy(V[:, sc, :], ps[:])
                    if h < H:
                        _ph2_unit(h, k)
                    if h >= 3:
                        _ph3_unit(h - 3, k)
                if h >= 3:
                    _ph3_tail(h - 3)
                if h < H:
                    _gamma(h)
                if h == 0:
                    # V projection (natural layout), after pipeline start
                    wv_t = wp.tile([P, NCHUNK, D], bf16, tag="w")
                    nc.sync.dma_start(
                        wv_t[:], wv.rearrange("(kc p) d -> p kc d", p=P))
                    for kc in range(NCHUNK):
                        t = vstp.tile([P, S], bf16, tag="vst")
                        nc.sync.dma_start(t[:], vT[kc * P:(kc + 1) * P, :])
                        vc.append(t)
                if h == 14:
                    wo_t = wp.tile([P, NCHUNK, D], bf16, tag="w")
                    nc.sync.dma_start(
                        wo_t[:], wo.rearrange("(kc p) d -> p kc d", p=P))

            # ---- output projection (natural [i, dout]) ----
            for ic in range(NCHUNK):
                ps = psA.tile([P, S], f32, tag="ps")
                for kc in range(NCHUNK):
                    for dt in range(2):
                        nc.tensor.matmul(
                            ps[:, dt * 512:(dt + 1) * 512],
                            XA[:, kc, ic * P:(ic + 1) * P],
                            wo_t[:, kc, dt * 512:(dt + 1) * 512],
                            start=(kc == 0), stop=(kc == NCHUNK - 1))
                xo = xop.tile([P, D], f32, tag="xo")
                nc.vector.tensor_copy(xo[:], ps[:])
                nc.sync.dma_start(xout_d[ic * P:(ic + 1) * P, :], xo[:])

    nc.finalize()
    return nc


def _layernorm_np(x, g, b):
    mu = x.mean(-1, keepdims=True)
    var = x.var(-1, keepdims=True)
    return (x - mu) / np.sqrt(var + LN_EPS) * g + b


def kernel(query, key, value, compability_u, code, W, b,
           q_wc_w, q_wc_b, q_ln_g, q_ln_b,
           k_wc_w, k_wc_b, k_ln_g, k_ln_b,
           v_wc_w, v_wc_b, v_ln_g, v_ln_b,
           o_wc_w, o_wc_b, o_ln_g, o_ln_b):
    global LAST_RESULT, _CACHED_NC

    query = np.asarray(query, dtype=np.float32)
    key = np.asarray(key, dtype=np.float32)
    value = np.asarray(value, dtype=np.float32)
    u = np.asarray(compability_u, dtype=np.float32)
    code = np.asarray(code, dtype=np.float32)
    W = np.asarray(W, dtype=np.float32)
    b = np.asarray(b, dtype=np.float32)
    if np.any(b != 0.0):
        raise NotImplementedError("nonzero projection bias b is not supported")

    weffT = {}
    for p, (wc_w, wc_b, ln_g, ln_b) in {
        "q": (q_wc_w, q_wc_b, q_ln_g, q_ln_b),
        "k": (k_wc_w, k_wc_b, k_ln_g, k_ln_b),
        "v": (v_wc_w, v_wc_b, v_ln_g, v_ln_b),
        "o": (o_wc_w, o_wc_b, o_ln_g, o_ln_b),
    }.items():
        tcv = _layernorm_np(code @ np.asarray(wc_w, np.float32).T
                            + np.asarray(wc_b, np.float32),
                            np.asarray(ln_g, np.float32),
                            np.asarray(ln_b, np.float32))
        weffT[p] = np.ascontiguousarray((W * tcv[None, :]).T).astype(
            ml_dtypes.bfloat16)

    if _CACHED_NC is None:
        _CACHED_NC = _build_bass()
    nc = _CACHED_NC

    in_maps = []
    with np.errstate(divide="ignore"):
        lnu_all = np.log(u)                       # [B, S]; -inf where u == 0
    for c in range(B):
        lnu = lnu_all[c]
        finite = lnu[np.isfinite(lnu)]
        mhat = float(np.median(finite)) - 7.0 if finite.size else -7.0
        biasj = ((lnu + (LOG1E6 + mhat)) * 0.5).reshape(NCHUNK, P).T  # [p, chunk]
        lnu8c = (8.0 * (lnu - mhat)).reshape(NCHUNK, P).T
        in_maps.append({
            "qT": np.ascontiguousarray(query[c].T).astype(ml_dtypes.bfloat16),
            "kT": np.ascontiguousarray(key[c].T).astype(ml_dtypes.bfloat16),
            "vT": np.ascontiguousarray(value[c].T).astype(ml_dtypes.bfloat16),
            "wq": weffT["q"], "wk": weffT["k"],
            "wv": weffT["v"], "wo": weffT["o"],
            "biasj": np.ascontiguousarray(biasj, dtype=np.float32),
            "lnu8c": np.ascontiguousarray(lnu8c, dtype=np.float32),
        })

    LAST_RESULT = run_bass_kernel_spmd(nc, in_maps, core_ids=list(range(B)))
    res = LAST_RESULT.results

    x = np.stack([res[c]["xout"] for c in range(B)])          # [B, S, D]
    attT = np.stack([res[c]["attT"] for c in range(B)])       # [B, H, j, i]
    attention = attT.transpose(0, 1, 3, 2)                    # [B, H, i, j] view
    return x, attention
